# revision 4
# baseline (speedup 1.0000x reference)
import sys
for p in ('/opt/trn_rl_repo', '/opt/pypackages'):
    if p not in sys.path:
        sys.path.insert(0, p)
import numpy as np
import concourse.bass as bass
import concourse.tile as tile
import concourse.bacc as bacc
from concourse import mybir
from concourse.bass_utils import run_bass_kernel_spmd

F32, F16 = mybir.dt.float32, mybir.dt.float16

BS = 16384
NCORES = 8
S = BS // NCORES          # 2048 samples per core
NT = S // 128             # 16 tiles of 128 samples
NCH = S // 512            # 4 chunks of 512 samples

NUM_ALLY, NUM_ADV, NUM_LM = 3, 2, 3
HID = 64

# branch geometry: (n_entities, d, quad_cols) ; strip width = 64 + d
BR = [("ally", 3, 4), ("adv", 2, 4), ("lm", 3, 2)]

# X column indices of entity features
def feat_cols(br, e):
    if br == "ally":
        return [10 + 2 * e, 11 + 2 * e, 20 + 2 * e, 21 + 2 * e]
    if br == "adv":
        return [16 + 2 * e, 17 + 2 * e, 26 + 2 * e, 27 + 2 * e]
    return [4 + 2 * e, 5 + 2 * e]

# X_f gathered column order (26 cols)
XF_COLS = []
for br, n, d in BR:
    for e in range(n):
        XF_COLS += feat_cols(br, e)
XF_OFF = {"ally": 0, "adv": 12, "lm": 20}

# zh layout (fp16 buffer, transposed -> Wprod rows)
# z-products region: per branch, per entity, per i: [64 k-products | d quad]
Z_ALLY = 0                      # 3 * 4 * 68 = 816
Z_ADV = 816                     # 2 * 4 * 68 = 544
Z_LM = 1360                     # 3 * 2 * 66 = 396
RS = 1756                       # r-strips: ally 3*68, adv 2*68, lm 3*66 = 538
RS_ADV = RS + 3 * 68
RS_LM = RS + 5 * 68
ZPAD = 2294
CT = 2304                       # 18 * 128
NZT = CT // 128                 # 18 row-tiles after transpose

_built = None


def _host_weights(inp):
    """Precompute all folded weight matrices in float64, return dict of arrays."""
    sm = np.asarray(inp["merger_w"], np.float64)[0]          # [4, 64]
    sm = np.exp(sm - sm.max(axis=0, keepdims=True))
    sm = sm / sm.sum(axis=0, keepdims=True)                  # softmax over HEAD

    Wx = np.zeros((31, HID))                                 # X^T-row contraction (fp32)
    Wprod = np.zeros((CT, HID))                              # zh^T-row contraction (fp16)
    W1big = np.zeros((31, 512))                              # h matmul rhs
    bias = np.zeros(HID)

    blk = 0
    for br, n, d in BR:
        W1 = np.asarray(inp[f"{br}_W1"], np.float64)         # [d, 64]
        b1 = np.asarray(inp[f"{br}_b1"], np.float64)
        W2 = np.asarray(inp[f"{br}_W2"], np.float64)         # [64, (d+1)*256]
        b2 = np.asarray(inp[f"{br}_b2"], np.float64)
        W2aug = np.concatenate([W2, b2[None]], 0)            # [65, (d+1)*256]
        Wp = W2aug.reshape(65, d + 1, 4, HID)                # [k65, i, head, j]
        Wm = np.einsum("kihj,hj->kij", Wp, sm)               # [65, d+1, 64]
        W1aug = np.concatenate([W1, b1[None]], 0)            # [d+1, 64]
        # quad: 0.01 * f_i * u_k * Wm[k,i,j], u = f_aug @ W1aug
        Q = 0.01 * np.einsum("lk,kij->ilj", W1aug, Wm[:64])  # [i(d+1), l(d+1), j]

        for e in range(n):
            fc = feat_cols(br, e)
            # h-matmul columns for this entity
            col0 = blk * 64
            for l in range(d):
                W1big[fc[l], col0:col0 + 64] = W1[l]
            W1big[30, col0:col0 + 64] = b1
            blk += 1

            # z-product region rows
            if br == "ally":
                base = Z_ALLY + e * 272
            elif br == "adv":
                base = Z_ADV + e * 272
            else:
                base = Z_LM + e * 132
            w = 64 + d
            for i in range(d):
                rb = base + i * w
                Wprod[rb:rb + 64] += 0.99 * Wm[:64, i]       # f_i * r_k
                Wprod[rb + 64:rb + 64 + d] += Q[i, :d]       # f_i * f_l quad
                # l = d (ones): 0.01*b1-path -> Wx feat row
                Wx[fc[i]] += Q[i, d]
                # k = 64 (b2 row): f_i * Wm[64, i]
                Wx[fc[i]] += Wm[64, i]
            # i = d terms: h_k * Wm[k, d, j]
            if br == "ally":
                sb = RS + e * 68
            elif br == "adv":
                sb = RS_ADV + e * 68
            else:
                sb = RS_LM + e * 66
            Wprod[sb:sb + 64] += 0.99 * Wm[:64, d]           # r_k part
            for l in range(d):                               # 0.01*u_k part
                Wx[fc[l]] += Q[d, l]
            bias += Q[d, d]                                  # 0.01*b1 part of i=d
            bias += Wm[64, d]                                # k=64, i=d constant

    # self head
    Wx[0:4] += np.asarray(inp["self_W"], np.float64)
    bias += np.asarray(inp["self_b"], np.float64)
    Wx[30] += bias

    fc1_W = np.asarray(inp["fc1_W"], np.float64)             # [66, 64]
    fc1_b = np.asarray(inp["fc1_b"], np.float64)
    fc1o = fc1_W[:64]                                        # [64, 64]
    fc1a = np.concatenate([fc1_W[64:66], fc1_b[None]], 0)    # [3, 64]
    fc2_W = np.asarray(inp["fc2_W"], np.float64)             # [64, 1]
    fc2_b = float(np.asarray(inp["fc2_b"]).reshape(-1)[0])

    return dict(
        w1big=W1big.astype(np.float32),
        wx=Wx.astype(np.float32),
        wprod=Wprod.reshape(NZT, 128, HID).astype(np.float16),
        fc1o=fc1o.astype(np.float16),
        fc1a=fc1a.astype(np.float32),
        fc2=fc2_W.astype(np.float32),
        fc2b=np.array([[fc2_b]], np.float32),
    )


def _build():
    nc = bacc.Bacc("TRN2", target_bir_lowering=False, debug=False)
    ALU = mybir.AluOpType
    ACTF = mybir.ActivationFunctionType

    xt_d = nc.dram_tensor("xt", [31, S], F32, kind="ExternalInput")
    xf_d = nc.dram_tensor("xf", [128, NT, 26], F32, kind="ExternalInput")
    at_d = nc.dram_tensor("at", [3, S], F32, kind="ExternalInput")
    w1big_d = nc.dram_tensor("w1big", [31, 512], F32, kind="ExternalInput")
    wx_d = nc.dram_tensor("wx", [31, HID], F32, kind="ExternalInput")
    wprod_d = nc.dram_tensor("wprod", [NZT, 128, HID], F16, kind="ExternalInput")
    fc1o_d = nc.dram_tensor("fc1o", [64, 64], F16, kind="ExternalInput")
    fc1a_d = nc.dram_tensor("fc1a", [3, 64], F32, kind="ExternalInput")
    fc2_d = nc.dram_tensor("fc2", [64, 1], F32, kind="ExternalInput")
    fc2b_d = nc.dram_tensor("fc2b", [1, 1], F32, kind="ExternalInput")
    xout_d = nc.dram_tensor("xout", [64, S], F32, kind="ExternalOutput")
    qout_d = nc.dram_tensor("qout", [1, S], F32, kind="ExternalOutput")

    with tile.TileContext(nc) as tc:
        with tc.tile_pool(name="const", bufs=1) as cst, \
             tc.tile_pool(name="zh", bufs=3) as zhp, \
             tc.tile_pool(name="ztb", bufs=2) as ztp, \
             tc.tile_pool(name="head", bufs=3) as hd, \
             tc.tile_pool(name="psh", bufs=2, space="PSUM") as pshp, \
             tc.tile_pool(name="pso", bufs=2, space="PSUM") as psop, \
             tc.tile_pool(name="psx", bufs=2, space="PSUM") as psxp, \
             tc.tile_pool(name="psq", bufs=2, space="PSUM") as psqp:

            xt = cst.tile([31, S], F32)
            xf = cst.tile([128, NT, 26], F32)
            at = cst.tile([3, S], F32)
            w1big = cst.tile([31, 512], F32)
            wx = cst.tile([31, HID], F32)
            wprod = cst.tile([128, NZT, HID], F16)
            fc1o = cst.tile([64, 64], F16)
            fc1a = cst.tile([3, 64], F32)
            fc2 = cst.tile([64, 1], F32)
            fc2b = cst.tile([1, 1], F32)
            nc.sync.dma_start(xt[:], xt_d[:])
            nc.sync.dma_start(xf[:], xf_d[:])
            nc.sync.dma_start(at[:], at_d[:])
            nc.sync.dma_start(w1big[:], w1big_d[:])
            nc.sync.dma_start(wx[:], wx_d[:])
            nc.sync.dma_start(wprod[:], wprod_d.rearrange("t p j -> p t j"))
            nc.sync.dma_start(fc1o[:], fc1o_d[:])
            nc.sync.dma_start(fc1a[:], fc1a_d[:])
            nc.sync.dma_start(fc2[:], fc2_d[:])
            nc.sync.dma_start(fc2b[:], fc2b_d[:])

            for ch in range(NCH):
                zt = ztp.tile([128, NZT, 512], F16)
                for tt in range(4):
                    ti = ch * 4 + tt
                    psh = pshp.tile([128, 512], F32)
                    nc.tensor.matmul(psh[:], lhsT=xt[:, ti * 128:(ti + 1) * 128],
                                     rhs=w1big[:], start=True, stop=True)

                    zh = zhp.tile([128, CT], F16)
                    # relu(u) into r-strips (fp16)
                    nc.scalar.activation(
                        zh[:, RS:RS + 340].rearrange("p (s k) -> p s k", s=5)[:, :, :64],
                        psh[:, 0:320].rearrange("p (s k) -> p s k", s=5),
                        ACTF.Relu)
                    nc.scalar.activation(
                        zh[:, RS_LM:RS_LM + 198].rearrange("p (s k) -> p s k", s=3)[:, :, :64],
                        psh[:, 320:512].rearrange("p (s k) -> p s k", s=3),
                        ACTF.Relu)
                    # quad feature copies into strip tails
                    xf_t = xf[:, ti, :]
                    nc.vector.tensor_copy(
                        zh[:, RS:RS + 340].rearrange("p (s k) -> p s k", s=5)[:, :, 64:68],
                        xf_t[:, 0:20].rearrange("p (s k) -> p s k", s=5))
                    nc.vector.tensor_copy(
                        zh[:, RS_LM:RS_LM + 198].rearrange("p (s k) -> p s k", s=3)[:, :, 64:66],
                        xf_t[:, 20:26].rearrange("p (s k) -> p s k", s=3))
                    nc.vector.memset(zh[:, ZPAD:CT], 0)

                    # products: out[(e,i,c)] = f_(e,i) * strip_e[c]
                    nc.vector.tensor_tensor(
                        zh[:, 0:816].rearrange("p (e i c) -> p e i c", e=3, i=4),
                        zh[:, RS:RS + 204].rearrange("p (e c) -> p e c", e=3)[:, :, None, :]
                            .broadcast_to([128, 3, 4, 68]),
                        xf_t[:, 0:12].rearrange("p (e i) -> p e i", e=3)[:, :, :, None]
                            .broadcast_to([128, 3, 4, 68]),
                        ALU.mult)
                    nc.vector.tensor_tensor(
                        zh[:, 816:1360].rearrange("p (e i c) -> p e i c", e=2, i=4),
                        zh[:, RS_ADV:RS_ADV + 136].rearrange("p (e c) -> p e c", e=2)[:, :, None, :]
                            .broadcast_to([128, 2, 4, 68]),
                        xf_t[:, 12:20].rearrange("p (e i) -> p e i", e=2)[:, :, :, None]
                            .broadcast_to([128, 2, 4, 68]),
                        ALU.mult)
                    nc.vector.tensor_tensor(
                        zh[:, 1360:1756].rearrange("p (e i c) -> p e i c", e=3, i=2),
                        zh[:, RS_LM:RS_LM + 198].rearrange("p (e c) -> p e c", e=3)[:, :, None, :]
                            .broadcast_to([128, 3, 2, 66]),
                        xf_t[:, 20:26].rearrange("p (e i) -> p e i", e=3)[:, :, :, None]
                            .broadcast_to([128, 3, 2, 66]),
                        ALU.mult)

                    nc.sync.dma_start_transpose(
                        zt[:, :, tt * 128:(tt + 1) * 128], zh[:])

                sl = slice(ch * 512, (ch + 1) * 512)
                pso = psop.tile([64, 512], F32)
                nc.tensor.matmul(pso[:], lhsT=wx[:], rhs=xt[:, sl],
                                 start=True, stop=False)
                for t in range(NZT):
                    nc.tensor.matmul(pso[:], lhsT=wprod[:, t, :], rhs=zt[:, t, :],
                                     start=False, stop=(t == NZT - 1))
                o64 = hd.tile([64, 512], F16)
                nc.scalar.activation(o64[:], pso[:], ACTF.Relu)

                psx = psxp.tile([64, 512], F32)
                nc.tensor.matmul(psx[:], lhsT=fc1o[:], rhs=o64[:],
                                 start=True, stop=False)
                nc.tensor.matmul(psx[:], lhsT=fc1a[:], rhs=at[:, sl],
                                 start=False, stop=True)
                xsb = hd.tile([64, 512], F32)
                nc.scalar.activation(xsb[:], psx[:], ACTF.Relu)
                nc.sync.dma_start(xout_d[:, sl], xsb[:])

                psq = psqp.tile([1, 512], F32)
                nc.tensor.matmul(psq[:], lhsT=fc2[:], rhs=xsb[:],
                                 start=True, stop=True)
                qsb = hd.tile([1, 512], F32)
                nc.scalar.activation(qsb[:], psq[:], ACTF.Identity, bias=fc2b[:])
                nc.sync.dma_start(qout_d[:, sl], qsb[:])

    nc.compile()
    return nc


def _get_nc():
    global _built
    if _built is None:
        _built = _build()
    return _built


def _in_maps(inputs):
    X = np.asarray(inputs["inputs"], np.float32)             # [BS, 30]
    A = np.asarray(inputs["actions"], np.float32)            # [BS, 2]
    W = _host_weights(inputs)
    maps = []
    for c in range(NCORES):
        Xc = X[c * S:(c + 1) * S]                            # [S, 30]
        Ac = A[c * S:(c + 1) * S]
        xt = np.concatenate([Xc.T, np.ones((1, S), np.float32)], 0)  # [31, S]
        xf = np.ascontiguousarray(
            Xc[:, XF_COLS].reshape(NT, 128, 26).transpose(1, 0, 2))  # [128, NT, 26]
        at = np.concatenate([Ac.T, np.ones((1, S), np.float32)], 0)  # [3, S]
        m = dict(xt=np.ascontiguousarray(xt), xf=xf, at=np.ascontiguousarray(at))
        m.update(W)
        maps.append(m)
    return maps


def kernel(**inputs):
    nc = _get_nc()
    maps = _in_maps(inputs)
    res = run_bass_kernel_spmd(nc, maps, core_ids=list(range(NCORES)))
    outs = res.results
    q = np.concatenate([outs[c]["qout"].reshape(S, 1) for c in range(NCORES)], 0)
    x = np.concatenate([outs[c]["xout"].T for c in range(NCORES)], 0)
    return q.astype(np.float32), x.astype(np.float32)
